# revision 2
# baseline (speedup 1.0000x reference)
"""Trainium2 Bass kernel for nn_ANN_LeafRiver_Sigmoid_qsim.

Problem: elementwise-recurrent RNN over B=500000 steps:
    seq = x[:, 0, :]                                  # [B, 16]
    y_t   = bias + seq[t] @ W + s_{t-1} * w_y         # [64]
    s_t   = sigmoid(y_t)
    out_t = bias_ln + s_t @ w_ln                      # scalar
returns (out[:, None], y_last[None, :], s_last[None, :]).

Parallelization: the step Jacobian is bounded by |sigmoid'| * |w_y| <= 0.25,
so the recurrence forgets its state at >= 4x per step.  We split B into many
independent chains, each warmed up for WARM steps on the preceding chain's
inputs starting from state 0; after WARM=16 steps the state error is
<= 0.25^16 ~ 2e-10 -- far below output tolerance.

Per core (8 cores, B/8 = 62500 steps each):
  2 "halves" (partition groups of 64 = H) x S=2 interleaved sets x C=512
  columns = 2048 chains of L=31 real steps (+16 warmup).
Per (set, step): one matmul computes a_k = bias + seq@W into a PSUM bank
(K=33 block-diagonal weights + a ones-row for the bias), a second matmul
accumulates diag(w_y) @ s_{k-1} on top, ACT applies sigmoid(PSUM)->SBUF,
and a readout matmul (w_ln block-diag, M=2) produces out_t values into a
PSUM bank that is evacuated by DVE and DMA'd out.
"""

import sys

if "/opt/trn_rl_repo" not in sys.path:
    sys.path.insert(0, "/opt/trn_rl_repo")

import numpy as np
import ml_dtypes

import concourse.bass as bass
import concourse.bacc as bacc
import concourse.mybir as mybir
from concourse.bass_utils import run_bass_kernel_spmd
from concourse.tile import TileContext

AF = mybir.ActivationFunctionType
BF16 = mybir.dt.bfloat16
F32 = mybir.dt.float32
NPBF16 = ml_dtypes.bfloat16

# ---- problem constants (hardcoded; kernel.py must be self-contained) ----
B, T, I, H = 500000, 4, 16, 64
NCORES = 8
BCORE = B // NCORES          # 62500
HALF = BCORE // 2            # 31250 per partition-half
WARM = 16                    # warmup steps per chain
C = 512                      # chains per (set, half) = matmul free dim
S = 2                        # interleaved sets (latency hiding)
L = 31                       # real steps per chain; C*S*L = 31744 >= HALF
SETCOLS = C * L              # 15872 b-values covered per (set, half)
KTOT = WARM + L              # 47 total steps per chain
NSEQ = SETCOLS + KTOT + 1    # columns per seq tile (max idx KTOT-1 + (C-1)*L)
NRO = S * L                  # readout matmuls per core (62)
NG = (NRO + 3) // 4          # PSUM readout groups of 4 (16)

_CACHED = {}


def _build_nc():
    if "nc" in _CACHED:
        return _CACHED["nc"]
    nc = bacc.Bacc(None)
    seq0_d = nc.declare_dram_parameter("seq0", [33, NSEQ], BF16, isOutput=False)
    seq1_d = nc.declare_dram_parameter("seq1", [33, NSEQ], BF16, isOutput=False)
    wmat_d = nc.declare_dram_parameter("wmat", [33, 128], BF16, isOutput=False)
    wdiag_d = nc.declare_dram_parameter("wdiag", [128, 128], BF16, isOutput=False)
    wro_d = nc.declare_dram_parameter("wro", [128, 2], BF16, isOutput=False)
    out_d = nc.declare_dram_parameter("out", [NG, 8, C], F32, isOutput=True)

    with TileContext(nc) as tc:
        with tc.tile_pool(name="const", bufs=1) as cpool, \
             tc.tile_pool(name="seq", bufs=1) as qpool, \
             tc.tile_pool(name="s_sb", bufs=6) as spool, \
             tc.tile_pool(name="stage", bufs=3) as gpool, \
             tc.tile_pool(name="psum_rec", bufs=4, space="PSUM") as rpool, \
             tc.tile_pool(name="psum_ro", bufs=3, space="PSUM") as opool:

            seq_sb = []
            for sd in (seq0_d, seq1_d):
                t = qpool.tile([33, NSEQ], BF16, tag=f"seq{len(seq_sb)}")
                nc.sync.dma_start(out=t[:, :], in_=sd[:, :])
                seq_sb.append(t)
            wmat = cpool.tile([33, 128], BF16, tag="wmat")
            nc.sync.dma_start(out=wmat[:, :], in_=wmat_d[:, :])
            wdiag = cpool.tile([128, 128], BF16, tag="wdiag")
            nc.sync.dma_start(out=wdiag[:, :], in_=wdiag_d[:, :])
            wro = cpool.tile([128, 2], BF16, tag="wro")
            nc.sync.dma_start(out=wro[:, :], in_=wro_d[:, :])

            s_prev = [None] * S
            ro_tile = None
            for k in range(KTOT):
                for s in range(S):
                    y_ps = rpool.tile([128, C], F32, tag="rec")
                    rhs = seq_sb[s][:, k : k + C * L : L]
                    nc.tensor.matmul(
                        y_ps[:, :], wmat[:, :], rhs,
                        start=True, stop=(k == 0),
                    )
                    if k > 0:
                        nc.tensor.matmul(
                            y_ps[:, :], wdiag[:, :], s_prev[s][:, :],
                            start=False, stop=True,
                        )
                    s_t = spool.tile([128, C], BF16, tag="s")
                    nc.scalar.activation(s_t[:, :], y_ps[:, :], AF.Sigmoid)
                    s_prev[s] = s_t

                    if k >= WARM:
                        r = (k - WARM) * S + s
                        g, i = divmod(r, 4)
                        if i == 0:
                            ro_tile = opool.tile([128, C], F32, tag="ro")
                        nc.tensor.matmul(
                            ro_tile[32 * i : 32 * i + 2, :], wro[:, :], s_t[:, :],
                            start=True, stop=True, tile_position=(0, 32 * i),
                        )
                        if i == 3 or r == NRO - 1:
                            stg = gpool.tile([128, C], F32, tag="stg")
                            nc.vector.tensor_copy(stg[:, :], ro_tile[:, :])
                            for ii in range(i + 1):
                                nc.sync.dma_start(
                                    out=out_d[g, 2 * ii : 2 * ii + 2, :],
                                    in_=stg[32 * ii : 32 * ii + 2, :],
                                )
    nc.finalize()
    _CACHED["nc"] = nc
    return nc


def _prep_inputs(x, weight, weight_y, bias, weight_ln, bias_ln):
    """Build per-core input maps (host-side shard + transpose + bf16)."""
    seq = np.ascontiguousarray(x[:, 0, :]).astype(np.float32)      # [B, 16]
    W = np.asarray(weight, np.float32)                              # [16, 64]
    wy = np.asarray(weight_y, np.float32).reshape(-1)               # [64]
    b = float(np.asarray(bias).reshape(-1)[0])
    wln = np.asarray(weight_ln, np.float32).reshape(-1)             # [64]

    # forcing rows for global b < 0 (core 0 chain 0 warmup): drive a <= -60
    # so sigmoid saturates to ~0, matching the true zero initial state.
    colsum = W.sum(axis=0)                                          # [64]
    F = (60.0 + abs(b)) / max(colsum.min(), 1e-3)
    seqpad = np.zeros((WARM + B + NSEQ, I), np.float32)
    seqpad[:WARM] = -F
    seqpad[WARM : WARM + B] = seq

    # weights
    wmat = np.zeros((33, 128), np.float32)
    wmat[:16, 0:64] = W
    wmat[16:32, 64:128] = W
    wmat[32, :] = b
    wdiag = np.zeros((128, 128), np.float32)
    np.fill_diagonal(wdiag[0:64, 0:64], wy)
    np.fill_diagonal(wdiag[64:128, 64:128], wy)
    wro = np.zeros((128, 2), np.float32)
    wro[0:64, 0] = wln
    wro[64:128, 1] = wln

    wmat16 = wmat.astype(NPBF16)
    wdiag16 = wdiag.astype(NPBF16)
    wro16 = wro.astype(NPBF16)

    in_maps = []
    for core in range(NCORES):
        m = {"wmat": wmat16, "wdiag": wdiag16, "wro": wro16}
        for s in range(S):
            buf = np.empty((33, NSEQ), np.float32)
            for h in range(2):
                lo = core * BCORE + h * HALF + s * SETCOLS  # global b of col 0 (- WARM offset in seqpad)
                sl = seqpad[lo : lo + NSEQ]                  # [NSEQ, 16]
                buf[16 * h : 16 * h + 16, :] = sl.T
            buf[32, :] = 1.0
            m[f"seq{s}"] = buf.astype(NPBF16)
        in_maps.append(m)
    return in_maps


def _assemble(results, bias_ln):
    """results[core]["out"] [NG, 8, C] -> h_n [B] (without bias_ln added yet)."""
    bln = float(np.asarray(bias_ln).reshape(-1)[0])
    h_n = np.empty(B, np.float32)
    for core in range(NCORES):
        out = results[core]["out"]                     # [NG, 8, C]
        # r = kr*S + s ; g,i = divmod(r,4); row = 2*i + h ; col = j
        # value is out for chain (h,s,j) at real step kr
        r_idx = np.arange(NRO)
        g = r_idx // 4
        i = r_idx % 4
        # A[r, h, j]
        A = out[g[:, None], (2 * i)[:, None] + np.arange(2)[None, :], :]
        A = A.reshape(L, S, 2, C)                      # [kr, s, h, j]
        for h in range(2):
            base = core * BCORE + h * HALF
            for s in range(S):
                flat = A[:, s, h, :].T.reshape(-1)     # [j, kr] -> j*L + kr
                n = min(SETCOLS, HALF - s * SETCOLS)
                h_n[base + s * SETCOLS : base + s * SETCOLS + n] = flat[:n]
    return h_n + bln


def _final_states(x, weight, weight_y, bias):
    """Exact final (y_h, y_hs) via short host-side warmup (contraction 4x/step)."""
    seq = np.asarray(x[B - 48 :, 0, :], np.float32)
    W = np.asarray(weight, np.float32)
    wy = np.asarray(weight_y, np.float32).reshape(-1)
    b = np.float32(np.asarray(bias).reshape(-1)[0])
    s = np.zeros(H, np.float32)
    y = np.zeros(H, np.float32)
    for t in range(seq.shape[0]):
        y = (b + seq[t] @ W + s * wy).astype(np.float32)
        s = (1.0 / (1.0 + np.exp(-y))).astype(np.float32)
    return y, s


def kernel(x, weight, weight_y, bias, weight_ln, bias_ln):
    nc = _build_nc()
    in_maps = _prep_inputs(x, weight, weight_y, bias, weight_ln, bias_ln)
    res = run_bass_kernel_spmd(nc, in_maps, core_ids=list(range(NCORES)))
    h_n = _assemble(res.results, bias_ln)
    y_f, s_f = _final_states(x, weight, weight_y, bias)
    return h_n[:, None], y_f[None, :], s_f[None, :]


if __name__ == "__main__":
    # quick self-check against a small numpy recurrence
    rng = np.random.default_rng(0)
    x = rng.standard_normal((B, T, I)).astype(np.float32)
    weight = rng.random((I, H), np.float32)
    weight_y = rng.random((1, H), np.float32)
    bias = rng.random((1,), np.float32)
    weight_ln = rng.random((H, 1), np.float32)
    bias_ln = rng.random((1,), np.float32)
    outs = kernel(x=x, weight=weight, weight_y=weight_y, bias=bias,
                  weight_ln=weight_ln, bias_ln=bias_ln)
    print([o.shape for o in outs])
